# revision 2
# baseline (speedup 1.0000x reference)
"""DA-RNN style encoder (LSTM + input attention) on 8 Trainium2 cores — v2.

Problem: nn_Encoder_63024350101963
  B=2048, T-1=31 steps, D=128 input feats, H=128 hidden.

v2 changes vs the fp32 baseline (252.8us):
  * bf16 data path everywhere except the cell-state chain and the sigmoid
    outputs (host ablation: sg must be fp32 — bf16 sg gives 4e-2 encoded
    error; everything-else-bf16 with fp32 sg gives 4.4e-3 vs the 2e-2 gate).
  * All matmuls bf16 (1 cyc/row at any N) — the baseline's fp32 W_hh
    matmuls ran at 4 cyc/row and dominated PE (1707ns/step -> 427ns/step).
  * bf16 I/O: input and both outputs are bf16 in HBM (host up/down-casts);
    DMA traffic 12.2MB -> 6.1MB per core.
  * Single bf16 bias matmul (K=1) per chunk per group; no hi/lo split.
  * Tanh table used directly for the c->h path (same table set as Sigmoid).

Layout: feature-on-partitions, batch-on-free throughout; host passes x
pre-transposed [D, T, B_local]; weights replicated; batch 2048 = 8 cores
x 256 rows, each core runs 2 independent 128-row subtile streams.

PSUM: two ping-pong tiles [128, 2048] (4 banks each); bank c = gate chunk
c (pytorch order i,f,g,o; g pre-scaled 2x so tanh(g) = 2*sigmoid(2g)-1)
holding [s0_t | s1_t | s0_t+1 | s1_t+1].
"""

import numpy as np

T = 31          # time steps (T_ref - 1)
D = 128         # input feature dim
H = 128         # hidden dim
G = 4 * H       # gate rows
NCORES = 8
B = 2048
BL = B // NCORES  # 256 batch rows per core
BS = 128          # batch sub-tile (2 per core)
NS = BL // BS
import os

WARM = int(os.environ.get("K4_WARM", "10"))  # PE warmup spins
T1_DVE = os.environ.get("K4_T1_DVE", "0") == "1"
HSPLIT = os.environ.get("K4_HSPLIT", "0") == "1"
# stream batch widths (must sum to BL=256); stream i lags i steps
BSS = [int(x) for x in os.environ.get("K4_BSS", "128,128").split(",")]
NSTR = len(BSS)
OFFS = [sum(BSS[:i]) for i in range(NSTR)]  # column offsets within BL

_CACHE = {}


def _build_program(loop_n=0):
    from contextlib import ExitStack

    import concourse.bacc as bacc
    import concourse.mybir as mybir
    import concourse.tile as tile

    dt = mybir.dt.float32
    bt = mybir.dt.bfloat16
    AF = mybir.ActivationFunctionType

    nc = bacc.Bacc("TRN2", target_bir_lowering=False, debug=False)

    xt_d = nc.dram_tensor("xt", [D, T, BL], bt, kind="ExternalInput").ap()
    # packed small tensors to minimize front DMA descriptor count:
    # fr = [ident | wxb_bf16]  [D, D+T]
    fr_d = nc.dram_tensor("fr", [D, D + T], bt, kind="ExternalInput").ap()
    # ww = [wih | whh]  [128, 2G]
    ww_d = nc.dram_tensor("ww", [128, 2 * G], bt, kind="ExternalInput").ap()
    # bo = [bias | ones-selector]  [2, 2H + 2BL]
    bo_d = nc.dram_tensor("bo", [2, 2 * H + 2 * BL], bt,
                          kind="ExternalInput").ap()

    wt_d = nc.dram_tensor("wt_out", [T, D, BL], bt, kind="ExternalOutput").ap()
    enc_d = nc.dram_tensor("enc_out", [T, H, BL], bt, kind="ExternalOutput").ap()

    with ExitStack() as ctx:
        tc = ctx.enter_context(tile.TileContext(nc))

        def body():
            _emit(nc, tc, ctx, mybir, dt, bt, AF,
                  xt_d, fr_d, ww_d, bo_d, wt_d, enc_d)

        if loop_n:
            with tc.For_i(0, loop_n, 1):
                body()
        else:
            body()

    nc.compile()
    return nc


def _emit(nc, tc, ctx, mybir, dt, bt, AF,
          xt_d, fr_d, ww_d, bo_d, wt_d, enc_d):
    from contextlib import ExitStack

    big = ctx.enter_context(tc.tile_pool(name="big", bufs=1))

    # ---- persistent SBUF tensors ----
    xt_s = big.tile([D, T * BL], bt, tag="xt")
    wid_s = big.tile([D, T * D], bt, tag="wid")
    wxb_s = big.tile([D, T], dt, tag="wxb")
    wxt_s = big.tile([D, T * BL], bt, tag="wxt")
    fr_s = big.tile([D, D + T], bt, tag="fr")       # [ident | wxb_bf16]
    ww_s = big.tile([128, 2 * G], bt, tag="ww")     # [wih | whh]
    bo_s = big.tile([2, 2 * H + 2 * BL], bt, tag="bo")
    zro_s = big.tile([H, BL], bt, tag="zro")      # h0 (both subtiles)
    czro_s = big.tile([H, 2 * BS], dt, tag="czro")  # c0 fp32
    attnT = big.tile([D, BL], bt, tag="attnT")

    ident_s = fr_s[:, 0:D]
    wih_s = ww_s[:, 0:G]
    whh_s = ww_s[:, G:2 * G]
    bias_s = bo_s[:, 0:2 * H]
    ones_s = bo_s[:, 2 * H:2 * H + 2 * BL]

    # DMA order drives the front critical path: tiny ident/wxb pack first
    # (unblocks the wid build), then the x stream, LSTM weights last (only
    # needed once the recurrence starts). One queue, minimal descriptors.
    nc.sync.dma_start(out=fr_s[:], in_=fr_d[:])
    xchunks = [(0, 10), (10, 20), (20, 28), (28, 31)]
    for t0, t1 in xchunks:
        nc.sync.dma_start(
            out=xt_s[:, t0 * BL:t1 * BL], in_=xt_d[:, t0:t1, :])
    nc.sync.dma_start(out=ww_s[:], in_=ww_d[:])
    nc.sync.dma_start(out=bo_s[:], in_=bo_d[:])
    # bf16 -> fp32 copy of the per-step attn weights (tensor_scalar needs a
    # float32 scalar operand)
    nc.vector.tensor_copy(wxb_s[:], fr_s[:, D:D + T])
    for t in range(T):
        nc.vector.tensor_scalar_mul(
            wid_s[:, t * D:(t + 1) * D], ident_s, wxb_s[:, t:t + 1])
    nc.vector.memset(zro_s[:], 0.0)
    nc.vector.memset(czro_s[:], 0.0)

    with ExitStack() as fctx:
        frs = fctx.enter_context(tc.tile_pool(name="fsmall", bufs=2))
        psf = fctx.enter_context(tc.tile_pool(name="psf", bufs=1, space="PSUM"))
        pst = fctx.enter_context(tc.tile_pool(name="pstr", bufs=2, space="PSUM"))

        # ---- x_score in natural [b, d]: ps_xs[j] += (xT_t chunk).T @ wid_t
        # PE warmup spins interleave with the chunked matmuls so the PE clock
        # ramps while waiting for x DMA chunks.
        pwm = pst.tile([D, D], bt, tag="warm")
        ps_xs = [psf.tile([BS, D], dt, tag=f"xs{j}", name=f"ps_xs{j}")
                 for j in range(NS)]
        for w in range(WARM):
            nc.tensor.transpose(pwm[:], ident_s, ident_s)
        for t in range(T):
            for j in range(NS):
                nc.tensor.matmul(
                    ps_xs[j][:],
                    lhsT=xt_s[:, t * BL + j * BS: t * BL + (j + 1) * BS],
                    rhs=wid_s[:, t * D:(t + 1) * D],
                    start=(t == 0),
                    stop=(t == T - 1),
                )

        # ---- softmax off PSUM in sigmoid form (exp(z) = s/(1-s) with
        # s = sigmoid(z)): only the sigmoid/tanh table is ever loaded, so
        # there is no act-table switch on the critical path.
        sxs = []
        for j in range(NS):
            nmx = frs.tile([BS, 1], dt, tag="nmx", name=f"nmx{j}")
            nc.vector.tensor_reduce(
                nmx[:], ps_xs[j][:], axis=mybir.AxisListType.X,
                op=mybir.AluOpType.max, negate=True,
            )
            sx = frs.tile([BS, D], dt, tag="sx", name=f"sx{j}")
            nc.scalar.activation(sx[:], ps_xs[j][:], AF.Sigmoid,
                                 bias=nmx[:])
            sxs.append(sx)
        for j in range(NS):
            sx = sxs[j]
            oms = frs.tile([BS, D], dt, tag="oms", name=f"oms{j}")
            nc.vector.tensor_scalar(
                oms[:], sx[:], -1.0, 1.0,
                op0=mybir.AluOpType.mult, op1=mybir.AluOpType.add)  # 1 - s
            rs = frs.tile([BS, D], dt, tag="rs", name=f"rs{j}")
            nc.vector.reciprocal(rs[:], oms[:])
            ex = frs.tile([BS, D], dt, tag="ex", name=f"ex{j}")
            sums = frs.tile([BS, 1], dt, tag="sums", name=f"sums{j}")
            nc.vector.tensor_mul(ex[:], sx[:], rs[:])
            nc.vector.tensor_reduce(
                sums[:], ex[:], axis=mybir.AxisListType.X,
                op=mybir.AluOpType.add,
            )
            rc = frs.tile([BS, 1], dt, tag="rc", name=f"rc{j}")
            nc.vector.reciprocal(rc[:], sums[:])
            at = frs.tile([BS, D], bt, tag="at", name=f"at{j}")
            nc.vector.tensor_scalar_mul(at[:], ex[:], rc[:])

            ptr2 = pst.tile([D, BS], bt, tag="ptr", name=f"ptr{j}")
            nc.tensor.transpose(ptr2[:], at[:], ident_s)
            nc.vector.tensor_copy(attnT[:, j * BS:(j + 1) * BS], ptr2[:])

    # ---- LSTM recurrence: NSTR subtile streams, stream i lags i steps ----
    # Each stream's serial chain (W_hh -> sigmoid -> c-update -> tanh -> h)
    # is the cycle floor; staggered streams keep every engine fed during the
    # other streams' cross-engine return legs.
    psg = ctx.enter_context(tc.tile_pool(name="psg", bufs=4, space="PSUM"))
    sgp = ctx.enter_context(tc.tile_pool(name="sg", bufs=12))
    sm = ctx.enter_context(tc.tile_pool(name="small", bufs=28))
    hst = ctx.enter_context(tc.tile_pool(name="hstage", bufs=3))
    jk = ctx.enter_context(tc.tile_pool(name="junk", bufs=8))

    c_prev = [czro_s[:, OFFS[s]:OFFS[s] + BSS[s]] for s in range(NSTR)]
    h_prev = [_Slice(zro_s[:, OFFS[s]:OFFS[s] + BSS[s]]) for s in range(NSTR)]
    hstages = {}
    sgs = {}      # (t, s) -> sg tile
    cs = {}       # (t, s) -> c tile
    ps_of = {}    # group idx -> psum tile

    def group_setup(t):
        nc.vector.tensor_mul(
            wxt_s[:, t * BL:(t + 1) * BL],
            xt_s[:, t * BL:(t + 1) * BL],
            attnT[:],
        )
        if t % 8 == 7 or t == T - 1:  # flush wt_out every 8 steps
            t0 = (t // 8) * 8
            t1 = min(t0 + 8, T)
            nc.sync.dma_start(
                out=wt_d[t0:t1].rearrange("t d b -> d t b"),
                in_=wxt_s[:, t0 * BL:t1 * BL].rearrange(
                    "d (t b) -> d t b", b=BL),
            )
        ps = psg.tile([128, 4 * BL], dt, tag="gates")
        ps_of[t] = ps
        # one start=True matmul per PSUM bank (= zero region = 2 chunks):
        # lhsT rows are the bank's two chunk biases, rhs selects the half.
        for bank in range(2):
            nc.tensor.matmul(
                ps[:, bank * 2 * BL:(bank + 1) * 2 * BL],
                lhsT=bias_s[:, bank * H:(bank + 1) * H],
                rhs=ones_s[:, 0:2 * BL], start=True, stop=False,
                skip_group_check=True,
            )
        for c in range(4):
            gseg = slice(c * H, (c + 1) * H)
            nc.tensor.matmul(
                ps[:, c * BL:(c + 1) * BL], lhsT=wih_s[:, gseg],
                rhs=wxt_s[:, t * BL:(t + 1) * BL], start=False, stop=False,
                skip_group_check=True,
            )

    def emit_whh(s, t):
        ps = ps_of[t]
        col = OFFS[s]
        bs = BSS[s]
        halves = ([(0, bs // 2), (bs // 2, bs)]
                  if HSPLIT and bs % 2 == 0 else [(0, bs)])
        for h0, h1 in halves:
            for c in range(4):
                gseg = slice(c * H, (c + 1) * H)
                nc.tensor.matmul(
                    ps[:, c * BL + col + h0: c * BL + col + h1],
                    lhsT=whh_s[:, gseg],
                    rhs=h_prev[s][:][:, h0:h1],
                    start=False,
                    stop=(s == NSTR - 1 and c in (1, 3) and h1 == bs),
                    skip_group_check=True,
                )

    def emit_sg(s, t):
        ps = ps_of[t]
        col = OFFS[s]
        bs = BSS[s]
        sg = sgp.tile([128, 4 * bs], dt, tag="sg", name=f"sg_{t}_{s}")
        ps_slot = ps[:].rearrange("p (c x) -> p c x", c=4)[
            :, :, col:col + bs]
        nc.scalar.activation(sg[:], ps_slot, AF.Sigmoid)
        sgs[(t, s)] = sg

    def emit_mid(s, t):
        bs = BSS[s]
        sg = sgs[(t, s)]
        si = sg[:, 0 * bs:1 * bs]
        sf = sg[:, 1 * bs:2 * bs]
        s2g = sg[:, 2 * bs:3 * bs]
        t2 = sm.tile([H, bs], dt, tag="t2", name=f"t2_{t}_{s}")
        j1 = jk.tile([H, 1], dt, tag="j1", name=f"j1_{t}_{s}")
        # t2 = tanh(g) * sigmoid(i) = (2*s2g - 1) * si
        nc.vector.affine_mul_reduce(
            out=t2[:], accum_out=j1[:], in0=s2g, in1=si,
            scale=2.0, bias=-1.0,
        )
        t1 = sm.tile([H, bs], dt, tag="t1", name=f"t1_{t}_{s}")
        eng = nc.vector if T1_DVE else nc.gpsimd
        eng.tensor_mul(t1[:], sf, c_prev[s][:])
        c_new = sm.tile([H, bs], dt, tag="c", name=f"c_{t}_{s}")
        nc.vector.tensor_add(c_new[:], t1[:], t2[:])
        c_prev[s] = c_new
        cs[(t, s)] = c_new

    def emit_back(s, t):
        bs = BSS[s]
        tc1 = sm.tile([H, bs], dt, tag="tc", name=f"tc_{t}_{s}")
        nc.scalar.activation(tc1[:], cs[(t, s)][:], AF.Tanh)
        so = sgs[(t, s)][:, 3 * bs:4 * bs]
        hstage = hstages[t // 4]
        base = (t % 4) * BL + OFFS[s]
        h_new = hstage[:, base:base + bs]
        if HSPLIT and bs % 2 == 0:
            hb = bs // 2
            nc.vector.tensor_mul(
                hstage[:, base:base + hb], tc1[:, 0:hb], so[:, 0:hb])
            nc.vector.tensor_mul(
                hstage[:, base + hb:base + bs], tc1[:, hb:bs], so[:, hb:bs])
        else:
            nc.vector.tensor_mul(h_new, tc1[:], so)
        h_prev[s] = _Slice(h_new)

    def enc_flush(twin):
        t0 = twin * 4
        n = min(4, T - t0)
        nc.sync.dma_start(
            out=enc_d[t0:t0 + n].rearrange("t h b -> h t b"),
            in_=hstages[twin][:].rearrange(
                "h (t b) -> h t b", t=4)[:, :n, :],
        )

    def active(t):
        return [s for s in range(NSTR) if 0 <= t - s < T]

    for t in range(T + NSTR - 1):
        for s in active(t):
            ts = t - s
            if s == 0:
                group_setup(ts)
            if s == 0 and ts % 4 == 0:
                hstages[ts // 4] = hst.tile([H, 4 * BL], bt, tag="hst",
                                            name=f"hst_{ts // 4}")
            emit_whh(s, ts)
            emit_sg(s, ts)
        for s in active(t):
            emit_mid(s, t - s)
        for s in active(t):
            ts = t - s
            emit_back(s, ts)
            if s == NSTR - 1 and ts % 4 == 3:
                enc_flush(ts // 4)
    if (T - 1) % 4 != 3:
        enc_flush((T - 1) // 4)


class _Slice:
    """Tiny adapter so h_prev[s][:] works for both tiles and AP slices."""

    def __init__(self, ap):
        self._ap = ap

    def __getitem__(self, key):
        return self._ap


def _get_program():
    if "nc" not in _CACHE:
        _CACHE["nc"] = _build_program()
    return _CACHE["nc"]


def _host_inputs(input_data, W_ih, W_hh, b_ih, b_hh, attn_w, attn_b):
    import ml_dtypes
    BF = ml_dtypes.bfloat16

    x = np.asarray(input_data, dtype=np.float32)
    W_ih = np.asarray(W_ih, dtype=np.float32)
    W_hh = np.asarray(W_hh, dtype=np.float32)
    b = (np.asarray(b_ih, dtype=np.float32)
         + np.asarray(b_hh, dtype=np.float32))
    w_x = np.asarray(attn_w, dtype=np.float32)[2 * H:]  # x-series part only

    # scale the g-gate block (pytorch order i,f,g,o -> rows 2H:3H) by 2
    # so tanh(g) = 2*sigmoid(2g) - 1 works with a single sigmoid pass.
    scale = np.ones((G, 1), np.float32)
    scale[2 * H:3 * H] = 2.0
    wih_t = np.ascontiguousarray((W_ih * scale).T).astype(BF)   # [D, 4H]
    whh_t = np.ascontiguousarray((W_hh * scale).T).astype(BF)   # [H, 4H]
    # [2, 2H]: row r, bank k, col h -> bias of chunk (2k + r)
    bias_m = np.ascontiguousarray(
        (b * scale[:, 0]).reshape(2, 2, H).transpose(1, 0, 2).reshape(2, 2 * H)
    ).astype(BF)

    wxb = np.tile(w_x[None, :], (D, 1)).astype(BF)  # [D, T]
    ident = np.eye(D, dtype=np.float32).astype(BF)
    # per-bank half selector: bank start-matmul rhs
    ones = np.zeros((2, 2 * BL), np.float32)
    ones[0, :BL] = 1.0
    ones[1, BL:] = 1.0
    ones = ones.astype(BF)

    fr = np.ascontiguousarray(np.concatenate([ident, wxb], axis=1))
    ww = np.ascontiguousarray(np.concatenate([wih_t, whh_t], axis=1))
    bo = np.ascontiguousarray(np.concatenate([bias_m, ones], axis=1))

    xb = x.astype(BF)
    in_maps = []
    for i in range(NCORES):
        xs = xb[i * BL:(i + 1) * BL]                  # [BL, T, D]
        xt = np.ascontiguousarray(xs.transpose(2, 1, 0))  # [D, T, BL]
        in_maps.append({
            "xt": xt,
            "fr": fr,
            "ww": ww,
            "bo": bo,
        })
    return in_maps


def _gather(results):
    weighted = np.empty((B, T, D), np.float32)
    encoded = np.empty((B, T, H), np.float32)
    for i, r in enumerate(results):
        # wt_out/enc_out are [T, D|H, BL] bf16 -> [BL, T, D|H] fp32
        weighted[i * BL:(i + 1) * BL] = (
            r["wt_out"].astype(np.float32).transpose(2, 0, 1))
        encoded[i * BL:(i + 1) * BL] = (
            r["enc_out"].astype(np.float32).transpose(2, 0, 1))
    return weighted, encoded


def kernel(input_data, W_ih, W_hh, b_ih, b_hh, attn_w, attn_b):
    from concourse.bass_utils import run_bass_kernel_spmd

    nc = _get_program()
    in_maps = _host_inputs(input_data, W_ih, W_hh, b_ih, b_hh, attn_w, attn_b)
    res = run_bass_kernel_spmd(nc, in_maps, list(range(NCORES)))
    return _gather(res.results)
